# revision 17
# baseline (speedup 1.0000x reference)
"""Multi-head causal attention (B=4,S=2048,D=768,H=12,HD=64) on 8 Trainium2 cores.

Sharding: 4-way head tensor-parallel (3 heads/core) x 2-way batch data-parallel
(2 batches/core).  Core c: batch group bg=c//4 (batches 2bg,2bg+1), head group
hg=c%4 (heads 3hg..3hg+2).

Per-core device program (SPMD; per-core differences come only from data):
  1. q/k projections emitted transposed (qT,kT: [64 head-dim partitions, rows]);
     v projection row-major with an appended ones column per head (softmax
     denominator rides along the AV matmul as psum row 64).
  2. Causal attention computed transposed: S_T[k,q] = kT.T @ qT, so P=exp(S_T)
     feeds AV directly with no P transpose.  Softmax skips the running max
     (scores are O(1) at this problem's scale; exp is mathematically identical
     to the reference since softmax is shift-invariant).  The two batches of a
     head run concurrently on the PE via 64-row tile packing.  AV accumulates
     ctxU_T[65, q512] = sum_k vE.T @ P_T (row 64 = denominator l).  Normalize:
     1/l via fast-approx DVE reciprocal, broadcast across partitions on GpSimd,
     one fused DVE multiply.
  3. Per 512-row q-block (x2 batches = 1024-row chunk): 8-core AllToAll (bf16,
     128-row shards) redistributes ctx so each core holds all 768 context
     features for its own 2x128 output rows; local projection with full Wp;
     bias via a K=1 ones-outer-product matmul.

Emission is software-pipelined: qk/v projection chunks are interleaved between
attention pairs as PE filler (keeps the PE dense so HAM stays at full clock),
and each chunk's out-projection is emitted one q-block later so the PE never
head-of-line blocks on the AllToAll round trip.

Host side only slices/casts/transposes inputs and concatenates output shards.
"""

import sys

if "/opt/trn_rl_repo" not in sys.path:
    sys.path.insert(0, "/opt/trn_rl_repo")

import numpy as np
import ml_dtypes

BF16 = ml_dtypes.bfloat16

B, S, D = 4, 2048, 768
H, HD = 12, 64
N_CORES = 8
BL = 2          # batches per core
HL = 3          # heads per core
R = BL * S      # 4096 rows per core
KSUB = D // 128  # 6

_CACHE = {}


def _build_nc():
    import concourse.bass as bass  # noqa: F401
    import concourse.tile as tile
    from concourse import bacc, mybir

    f32 = mybir.dt.float32
    bf16 = mybir.dt.bfloat16
    EXP = mybir.ActivationFunctionType.Exp

    nc = bacc.Bacc("TRN2", target_bir_lowering=False, debug=False,
                   num_devices=N_CORES)

    xT_d = nc.dram_tensor("xT", [D, R], bf16, kind="ExternalInput").ap()
    wqk_d = nc.dram_tensor("wqk", [D, 2 * HL * HD], bf16, kind="ExternalInput").ap()
    wv_d = nc.dram_tensor("wv", [D, HL * HD], bf16, kind="ExternalInput").ap()
    wp_d = nc.dram_tensor("wp", [D, D], bf16, kind="ExternalInput").ap()
    bp_d = nc.dram_tensor("bp", [1, D], bf16, kind="ExternalInput").ap()
    mk_d = nc.dram_tensor("mk", [2, 128, 1024], bf16, kind="ExternalInput").ap()
    out_d = nc.dram_tensor("out", [4, 2, 128, D], f32, kind="ExternalOutput").ap()

    RG = [[0, 1, 2, 3, 4, 5, 6, 7]]

    with tile.TileContext(nc) as tc:
        with tc.tile_pool(name="persist", bufs=1) as per, \
             tc.tile_pool(name="dram", bufs=1, space="DRAM") as dram, \
             tc.tile_pool(name="mix_ps", bufs=1, space="PSUM") as mix_ps, \
             tc.tile_pool(name="st_ps", bufs=2, space="PSUM") as st_ps, \
             tc.tile_pool(name="av_ps", bufs=3, space="PSUM") as av_ps, \
             tc.tile_pool(name="pt", bufs=6) as ptp, \
             tc.tile_pool(name="sm", bufs=4) as sm, \
             tc.tile_pool(name="agp", bufs=2) as agp, \
             tc.tile_pool(name="outp", bufs=2) as outp:
            # ---- persistent SBUF tensors -------------------------------
            wqk = per.tile([128, KSUB, 2 * HL * HD], bf16, tag="wqk")
            wv = per.tile([128, KSUB, HL * HD], bf16, tag="wv")
            xT = per.tile([128, KSUB, R], bf16, tag="xT")
            xTr = xT_d.rearrange("(o p) r -> p o r", p=128)
            for j in range(KSUB):  # consumption order for fast PE start
                nc.sync.dma_start(
                    wqk[:, j], wqk_d.rearrange("(o p) c -> p o c", p=128)[:, j])
                nc.sync.dma_start(xT[:, j, 0:512], xTr[:, j, 0:512])
                nc.sync.dma_start(
                    wv[:, j], wv_d.rearrange("(o p) c -> p o c", p=128)[:, j])
            for rc in range(1, 8):
                for j in range(KSUB):
                    nc.sync.dma_start(xT[:, j, rc * 512:rc * 512 + 512],
                                      xTr[:, j, rc * 512:rc * 512 + 512])
            masks = per.tile([128, 2, 1024], bf16, tag="mk")
            nc.sync.dma_start(masks[:], mk_d.rearrange("o p c -> p o c"))
            wp = per.tile([128, KSUB, D], bf16, tag="wp")
            nc.sync.dma_start(wp[:], wp_d.rearrange("(o p) c -> p o c", p=128))
            bp_sb = per.tile([1, D], bf16, tag="bp")
            nc.sync.dma_start(bp_sb[:], bp_d[:])
            onesP = per.tile([1, 128], bf16, tag="onesP")
            nc.vector.memset(onesP[:], 1.0)

            # pair p = head p; partitions 0:64 = batch 0, 64:128 = batch 1
            qT = per.tile([128, HL, S], bf16, tag="qT")
            kT = per.tile([128, HL, S], bf16, tag="kT")
            vE = per.tile([128, 2 * 16, HL * 65], bf16, tag="vE")
            nc.vector.memset(vE[:], 1.0)

            warm_in = dram.tile([8 * 192, 8], bf16, tag="warm_in")
            warm_out = dram.tile([8 * 192, 8], bf16, tag="warm_out")
            a2a_in = [dram.tile([8 * 192, 128], bf16, name=f"a2ai{qb}",
                                tag=f"a2ai{qb}") for qb in range(4)]
            a2a_out = [dram.tile([8 * 192, 128], bf16, name=f"a2ao{qb}",
                                 tag=f"a2ao{qb}") for qb in range(4)]

            # ---- emission helpers --------------------------------------
            def emit_qk_ct(b, rc, ct):
                r0 = b * S + rc * 512
                if True:
                    ps = mix_ps.tile([128, 512], f32, tag="mix", name="ps")
                    for j in range(KSUB):
                        nc.tensor.matmul(
                            ps[:],
                            lhsT=wqk[:, j, ct * 128:(ct + 1) * 128],
                            rhs=xT[:, j, r0:r0 + 512],
                            start=(j == 0), stop=(j == KSUB - 1))
                    for half in range(2):
                        gid = 2 * ct + half
                        dest = qT if gid < 3 else kT
                        pair = gid % 3
                        nc.vector.tensor_copy(
                            dest[b * 64:(b + 1) * 64, pair,
                                 rc * 512:(rc + 1) * 512],
                            ps[half * 64:(half + 1) * 64, :])

            def emit_qk_chunk(b, rc):
                for ct in range(3):
                    emit_qk_ct(b, rc, ct)

            def emit_v_tile(b, rt):
                r0 = b * S + rt * 128
                psv = mix_ps.tile([128, HL * HD], f32, tag="mix", name="psv")
                for j in range(KSUB):
                    nc.tensor.matmul(
                        psv[:], lhsT=xT[:, j, r0:r0 + 128], rhs=wv[:, j, :],
                        start=(j == 0), stop=(j == KSUB - 1))
                for h in range(HL):
                    nc.vector.tensor_copy(
                        vE[:, b * 16 + rt, h * 65:h * 65 + 64],
                        psv[:, h * 64:(h + 1) * 64])

            def emit_attn_pair(qb, pair, drain):
                n_k = 4 * (qb + 1)
                n_kp = n_k // 2
                q0 = qb * 512
                avs = [av_ps.tile([65, 512], f32, tag="av", name=f"av{u}")
                       for u in range(2)]
                for kp in range(n_kp):
                    drain(kp)  # due-units for this kp must precede it
                    o = kp - (n_kp - 2)  # diag pair offset; >=0 on diagonal
                    qv0 = 256 if o == 1 else 0  # valid q starts here
                    stps = [st_ps.tile([128, 2, 512], f32, tag="st",
                                       name=f"st{u}") for u in range(2)]
                    for i in range(2):
                        for u in range(2):
                            kt = 2 * kp + i
                            nc.tensor.matmul(
                                stps[u][:, i, qv0:512],
                                lhsT=kT[u * 64:(u + 1) * 64, pair,
                                        kt * 128:(kt + 1) * 128],
                                rhs=qT[u * 64:(u + 1) * 64, pair,
                                       q0 + qv0:q0 + 512],
                                start=True, stop=True)
                    for u in range(2):
                        pt = ptp.tile([128, 2, 512], bf16, tag="pt")
                        if qv0:
                            nc.vector.memset(pt[:, :, 0:qv0], 0.0)
                        nc.scalar.activation(pt[:, :, qv0:512],
                                             stps[u][:, :, qv0:512], EXP,
                                             scale=float(HD) ** -0.5)
                        if o >= 0:
                            mk2 = masks[:, o, :].rearrange("p (i c) -> p i c",
                                                           i=2)
                            nc.vector.tensor_mul(pt[:, :, qv0:512],
                                                 pt[:, :, qv0:512],
                                                 mk2[:, :, qv0:512])
                        for i in range(2):
                            kt = 2 * kp + i
                            nc.tensor.matmul(
                                avs[u][:],
                                lhsT=vE[:, u * 16 + kt,
                                        pair * 65:(pair + 1) * 65],
                                rhs=pt[:, i, :],
                                start=(kp == 0 and i == 0),
                                stop=(kp == n_kp - 1 and i == 1))
                lsbs, recs, bcss, ctxns = [], [], [], []
                for u in range(2):
                    lsbs.append(sm.tile([1, 512], f32, tag="lsb",
                                        name=f"lsb{u}"))
                    nc.vector.tensor_copy(lsbs[u][:], avs[u][64:65, :])
                for u in range(2):
                    recs.append(sm.tile([1, 512], f32, tag="rec",
                                        name=f"rec{u}"))
                    nc.vector.reciprocal_approx_fast(recs[u][:], lsbs[u][:])
                for u in range(2):
                    bcss.append(sm.tile([64, 512], f32, tag="bcs",
                                        name=f"bcs{u}"))
                    nc.gpsimd.partition_broadcast(bcss[u][:], recs[u][:])
                for u in range(2):
                    ctxn = sm.tile([64, 512], bf16, tag="ctxn",
                                   name=f"ctxn{u}")
                    ctxns.append(ctxn)
                    nc.vector.tensor_mul(ctxn[:], avs[u][0:64, :], bcss[u][:])
                    a2v = a2a_in[qb].rearrange("(j f) c -> f j c", f=192)
                    nc.sync.dma_start(
                        a2v[64 * pair:64 * (pair + 1), 4 * u:4 * u + 4, :],
                        ctxns[u].rearrange("p (q c) -> p q c", q=4))

            def emit_a2a(qb):
                nc.gpsimd.collective_compute(
                    "AllToAll", mybir.AluOpType.bypass,
                    ins=[a2a_in[qb][:]], outs=[a2a_out[qb][:]],
                    replica_groups=RG)

            def emit_outproj_blk(qb, blk, ag):
                if blk == 0:
                    nc.sync.dma_start(
                        ag[:], a2a_out[qb].rearrange("(o p) r -> p o r", p=128))
                if True:
                    osb = outp.tile([128, D], f32, tag="osb")
                    for nh in range(2):
                        po = mix_ps.tile([128, 384], f32, tag="mix", name="po")
                        n0 = nh * 384
                        for j in range(KSUB):
                            nc.tensor.matmul(po[:],
                                             lhsT=ag[:, blk * KSUB + j, :],
                                             rhs=wp[:, j, n0:n0 + 384],
                                             start=(j == 0), stop=False)
                        nc.tensor.matmul(po[:], lhsT=onesP[:],
                                         rhs=bp_sb[:, n0:n0 + 384],
                                         start=False, stop=True)
                        nc.vector.tensor_copy(osb[:, n0:n0 + 384], po[:])
                    nc.sync.dma_start(out_d[qb, blk], osb[:])

            # ---- software-pipelined emission ---------------------------
            # warmup collective: absorb ncfw first-call overhead during proj
            nc.sync.dma_start(warm_in[0:128, :], masks[:, 0, 0:8])
            nc.gpsimd.collective_compute(
                "AllToAll", mybir.AluOpType.bypass,
                ins=[warm_in[:]], outs=[warm_out[:]], replica_groups=RG)
            # prologue: everything attention qb0 needs
            emit_qk_chunk(0, 0)
            emit_qk_chunk(1, 0)
            for rt in range(4):
                emit_v_tile(0, rt)
                emit_v_tile(1, rt)

            # filler queue: (deadline=(qb,pair,kp), emit_fn), kept in
            # deadline order; before each k-pair all units due by then are
            # drained (hard ordering requirement: a unit must be emitted
            # before the attention that consumes its output), plus one unit
            # opportunistically per k-pair to spread PE filler.
            from collections import deque
            fq = deque()
            for rc in range(1, 4):
                for b in range(2):
                    for ct in range(3):
                        # qT/kT rows rc needed from (qb=rc, pair0, kp0)
                        fq.append(((rc, 0, 0), lambda b=b, rc=rc, ct=ct:
                                   emit_qk_ct(b, rc, ct)))
                for rt in range(4 * rc, 4 * rc + 4):
                    for b in range(2):
                        # vE row-tile rt consumed at kp=rt//2 of (qb=rc,pair0)
                        dl = (rc, 0, max(0, rt // 2 - 1))
                        fq.append((dl, lambda b=b, rt=rt: emit_v_tile(b, rt)))
            fq = deque(sorted(fq, key=lambda t: t[0]))

            def drain(n, due=None):
                k = 0
                while fq and (k < n or (due and fq[0][0] <= due)):
                    fq.popleft()[1]()
                    k += 1

            for qb in range(4):
                for pair in range(HL):
                    emit_attn_pair(qb, pair, lambda kp, qb=qb, pair=pair:
                                   drain(1, due=(qb, pair, kp)))
                    if qb >= 1 and pair == (0 if qb >= 2 else 1):
                        # previous chunk's out-proj; A2A(qb-1) done by now
                        ag = agp.tile([128, 2 * KSUB, 128], bf16, tag="ag",
                                      name=f"ag{qb}")
                        fq.append(((qb, pair, 98), lambda q=qb - 1, a=ag:
                                   emit_outproj_blk(q, 0, a)))
                        fq.append(((qb, pair, 99), lambda q=qb - 1, a=ag:
                                   emit_outproj_blk(q, 1, a)))
                emit_a2a(qb)
            drain(99)
            ag3 = agp.tile([128, 2 * KSUB, 128], bf16, tag="ag", name="ag3")
            emit_outproj_blk(3, 0, ag3)
            emit_outproj_blk(3, 1, ag3)

    nc.compile()
    return nc


def _get_nc():
    if "nc" not in _CACHE:
        _CACHE["nc"] = _build_nc()
    return _CACHE["nc"]


def _masks_np():
    k = np.arange(128)[:, None]
    q = np.arange(512)[None, :]
    tiles = [(q >= k + 128 * t) for t in range(4)]
    m = np.stack([np.concatenate([tiles[2 * o], tiles[2 * o + 1]], axis=1)
                  for o in range(2)])
    return m.astype(BF16)


def _prep_in_maps(x, Wq, Wk, Wv, Wp, bp):
    x = np.asarray(x, dtype=np.float32)
    mk = _masks_np()
    wp_full = np.asarray(Wp).astype(BF16)
    bp_row = np.asarray(bp, dtype=np.float32).reshape(1, D).astype(BF16)
    xT_bg = []
    for bg in range(2):
        xl = x[2 * bg:2 * bg + 2].reshape(R, D)
        xT_bg.append(np.ascontiguousarray(xl.T).astype(BF16))
    wqk_hg, wv_hg = [], []
    for hg in range(4):
        hs = slice(192 * hg, 192 * (hg + 1))
        wqk_hg.append(np.concatenate(
            [np.asarray(Wq)[:, hs], np.asarray(Wk)[:, hs]], axis=1).astype(BF16))
        wv_hg.append(np.asarray(Wv)[:, hs].astype(BF16))
    in_maps = []
    for c in range(N_CORES):
        bg, hg = c // 4, c % 4
        in_maps.append({
            "xT": xT_bg[bg],
            "wqk": wqk_hg[hg],
            "wv": wv_hg[hg],
            "wp": wp_full,
            "bp": bp_row,
            "mk": mk,
        })
    return in_maps


def kernel(x, Wq, Wk, Wv, Wp, bp):
    from concourse import bass_utils

    nc = _get_nc()
    in_maps = _prep_in_maps(x, Wq, Wk, Wv, Wp, bp)
    res = bass_utils.run_bass_kernel_spmd(nc, in_maps,
                                          core_ids=list(range(N_CORES)))
    out = np.empty((B, S, D), np.float32)
    for c in range(N_CORES):
        sh = res.results[c]["out"]  # [4 chunks, 2 blocks, 128, D]
        for qb in range(4):
            for blk in range(2):
                batch = 2 * blk + c // 4
                s0 = 512 * qb + 128 * (c % 4)
                out[batch, s0:s0 + 128] = sh[qb, blk]
    return out


# revision 18
# speedup vs baseline: 1.0446x; 1.0446x over previous
"""Multi-head causal attention (B=4,S=2048,D=768,H=12,HD=64) on 8 Trainium2 cores.

Sharding: 4-way head tensor-parallel (3 heads/core) x 2-way batch data-parallel
(2 batches/core).  Core c: batch group bg=c//4 (batches 2bg,2bg+1), head group
hg=c%4 (heads 3hg..3hg+2).

Per-core device program (SPMD; per-core differences come only from data):
  1. q/k projections emitted transposed (qT,kT: [64 head-dim partitions, rows]);
     v projection row-major with an appended ones column per head (softmax
     denominator rides along the AV matmul as psum row 64).
  2. Causal attention computed transposed: S_T[k,q] = kT.T @ qT, so P=exp(S_T)
     feeds AV directly with no P transpose.  Softmax skips the running max
     (scores are O(1) at this problem's scale; exp is mathematically identical
     to the reference since softmax is shift-invariant).  The two batches of a
     head run concurrently on the PE via 64-row tile packing.  AV accumulates
     ctxU_T[65, q512] = sum_k vE.T @ P_T (row 64 = denominator l).  Normalize:
     1/l via fast-approx DVE reciprocal, broadcast across partitions on GpSimd,
     one fused DVE multiply.
  3. Per 512-row q-block (x2 batches = 1024-row chunk): 8-core AllToAll (bf16,
     128-row shards) redistributes ctx so each core holds all 768 context
     features for its own 2x128 output rows; local projection with full Wp;
     bias via a K=1 ones-outer-product matmul.

Emission is software-pipelined: qk/v projection chunks are interleaved between
attention pairs as PE filler (keeps the PE dense so HAM stays at full clock),
and each chunk's out-projection is emitted one q-block later so the PE never
head-of-line blocks on the AllToAll round trip.

Host side only slices/casts/transposes inputs and concatenates output shards.
"""

import sys

if "/opt/trn_rl_repo" not in sys.path:
    sys.path.insert(0, "/opt/trn_rl_repo")

import numpy as np
import ml_dtypes

BF16 = ml_dtypes.bfloat16

B, S, D = 4, 2048, 768
H, HD = 12, 64
N_CORES = 8
BL = 2          # batches per core
HL = 3          # heads per core
R = BL * S      # 4096 rows per core
KSUB = D // 128  # 6

_CACHE = {}


def _build_nc():
    import concourse.bass as bass  # noqa: F401
    import concourse.tile as tile
    from concourse import bacc, mybir

    f32 = mybir.dt.float32
    bf16 = mybir.dt.bfloat16
    EXP = mybir.ActivationFunctionType.Exp

    nc = bacc.Bacc("TRN2", target_bir_lowering=False, debug=False,
                   num_devices=N_CORES)

    xT_d = nc.dram_tensor("xT", [D, R], bf16, kind="ExternalInput").ap()
    wqk_d = nc.dram_tensor("wqk", [D, 2 * HL * HD], bf16, kind="ExternalInput").ap()
    wv_d = nc.dram_tensor("wv", [D, HL * HD], bf16, kind="ExternalInput").ap()
    wp_d = nc.dram_tensor("wp", [D, D], bf16, kind="ExternalInput").ap()
    bp_d = nc.dram_tensor("bp", [1, D], bf16, kind="ExternalInput").ap()
    mk_d = nc.dram_tensor("mk", [2, 128, 1024], bf16, kind="ExternalInput").ap()
    out_d = nc.dram_tensor("out", [4, 2, 128, D], f32, kind="ExternalOutput").ap()

    RG = [[0, 1, 2, 3, 4, 5, 6, 7]]

    with tile.TileContext(nc) as tc:
        with tc.tile_pool(name="persist", bufs=1) as per, \
             tc.tile_pool(name="dram", bufs=1, space="DRAM") as dram, \
             tc.tile_pool(name="mix_ps", bufs=2, space="PSUM") as mix_ps, \
             tc.tile_pool(name="st_ps", bufs=2, space="PSUM") as st_ps, \
             tc.tile_pool(name="av_ps", bufs=2, space="PSUM") as av_ps, \
             tc.tile_pool(name="pt", bufs=6) as ptp, \
             tc.tile_pool(name="sm", bufs=4) as sm, \
             tc.tile_pool(name="agp", bufs=2) as agp, \
             tc.tile_pool(name="outp", bufs=2) as outp:
            # ---- persistent SBUF tensors -------------------------------
            wqk = per.tile([128, KSUB, 2 * HL * HD], bf16, tag="wqk")
            wv = per.tile([128, KSUB, HL * HD], bf16, tag="wv")
            xT = per.tile([128, KSUB, R], bf16, tag="xT")
            xTr = xT_d.rearrange("(o p) r -> p o r", p=128)
            for j in range(KSUB):  # consumption order for fast PE start
                nc.sync.dma_start(
                    wqk[:, j], wqk_d.rearrange("(o p) c -> p o c", p=128)[:, j])
                nc.sync.dma_start(xT[:, j, 0:512], xTr[:, j, 0:512])
                nc.sync.dma_start(
                    wv[:, j], wv_d.rearrange("(o p) c -> p o c", p=128)[:, j])
            for rc in range(1, 8):
                for j in range(KSUB):
                    nc.sync.dma_start(xT[:, j, rc * 512:rc * 512 + 512],
                                      xTr[:, j, rc * 512:rc * 512 + 512])
            masks = per.tile([128, 2, 1024], bf16, tag="mk")
            nc.sync.dma_start(masks[:], mk_d.rearrange("o p c -> p o c"))
            wp = per.tile([128, KSUB, D], bf16, tag="wp")
            nc.sync.dma_start(wp[:], wp_d.rearrange("(o p) c -> p o c", p=128))
            bp_sb = per.tile([1, D], bf16, tag="bp")
            nc.sync.dma_start(bp_sb[:], bp_d[:])
            onesP = per.tile([1, 128], bf16, tag="onesP")
            nc.vector.memset(onesP[:], 1.0)

            # pair p = head p; partitions 0:64 = batch 0, 64:128 = batch 1
            qT = per.tile([128, HL, S], bf16, tag="qT")
            kT = per.tile([128, HL, S], bf16, tag="kT")
            vE = per.tile([128, 2 * 16, HL * 65], bf16, tag="vE")
            nc.vector.memset(vE[:], 1.0)

            warm_in = dram.tile([8 * 192, 8], bf16, tag="warm_in")
            warm_out = dram.tile([8 * 192, 8], bf16, tag="warm_out")
            a2a_in = [dram.tile([8 * 192, 128], bf16, name=f"a2ai{qb}",
                                tag=f"a2ai{qb}") for qb in range(4)]
            a2a_out = [dram.tile([8 * 192, 128], bf16, name=f"a2ao{qb}",
                                 tag=f"a2ao{qb}") for qb in range(4)]

            # ---- emission helpers --------------------------------------
            def emit_qk_ct(b, rc, ct):
                r0 = b * S + rc * 512
                if True:
                    ps = mix_ps.tile([128, 512], f32, tag="mix", name="ps")
                    for j in range(KSUB):
                        nc.tensor.matmul(
                            ps[:],
                            lhsT=wqk[:, j, ct * 128:(ct + 1) * 128],
                            rhs=xT[:, j, r0:r0 + 512],
                            start=(j == 0), stop=(j == KSUB - 1))
                    for half in range(2):
                        gid = 2 * ct + half
                        dest = qT if gid < 3 else kT
                        pair = gid % 3
                        nc.vector.tensor_copy(
                            dest[b * 64:(b + 1) * 64, pair,
                                 rc * 512:(rc + 1) * 512],
                            ps[half * 64:(half + 1) * 64, :])

            def emit_qk_chunk(b, rc):
                for ct in range(3):
                    emit_qk_ct(b, rc, ct)

            def emit_v_tile(b, rt):
                r0 = b * S + rt * 128
                psv = mix_ps.tile([128, HL * HD], f32, tag="mix", name="psv")
                for j in range(KSUB):
                    nc.tensor.matmul(
                        psv[:], lhsT=xT[:, j, r0:r0 + 128], rhs=wv[:, j, :],
                        start=(j == 0), stop=(j == KSUB - 1))
                for h in range(HL):
                    nc.vector.tensor_copy(
                        vE[:, b * 16 + rt, h * 65:h * 65 + 64],
                        psv[:, h * 64:(h + 1) * 64])

            def emit_attn_pair(qb, pair, drain):
                n_k = 4 * (qb + 1)
                n_kp = n_k // 2
                q0 = qb * 512
                avs = [av_ps.tile([65, 512], f32, tag="av", name=f"av{u}")
                       for u in range(2)]
                for kp in range(n_kp):
                    drain(kp)  # due-units for this kp must precede it
                    o = kp - (n_kp - 2)  # diag pair offset; >=0 on diagonal
                    qv0 = 256 if o == 1 else 0  # valid q starts here
                    stps = [st_ps.tile([128, 2, 512], f32, tag="st",
                                       name=f"st{u}") for u in range(2)]
                    for i in range(2):
                        for u in range(2):
                            kt = 2 * kp + i
                            nc.tensor.matmul(
                                stps[u][:, i, qv0:512],
                                lhsT=kT[u * 64:(u + 1) * 64, pair,
                                        kt * 128:(kt + 1) * 128],
                                rhs=qT[u * 64:(u + 1) * 64, pair,
                                       q0 + qv0:q0 + 512],
                                start=True, stop=True)
                    for u in range(2):
                        pt = ptp.tile([128, 2, 512], bf16, tag="pt")
                        if qv0:
                            nc.vector.memset(pt[:, :, 0:qv0], 0.0)
                        nc.scalar.activation(pt[:, :, qv0:512],
                                             stps[u][:, :, qv0:512], EXP,
                                             scale=float(HD) ** -0.5)
                        if o >= 0:
                            mk2 = masks[:, o, :].rearrange("p (i c) -> p i c",
                                                           i=2)
                            nc.vector.tensor_mul(pt[:, :, qv0:512],
                                                 pt[:, :, qv0:512],
                                                 mk2[:, :, qv0:512])
                        for i in range(2):
                            kt = 2 * kp + i
                            nc.tensor.matmul(
                                avs[u][:],
                                lhsT=vE[:, u * 16 + kt,
                                        pair * 65:(pair + 1) * 65],
                                rhs=pt[:, i, :],
                                start=(kp == 0 and i == 0),
                                stop=(kp == n_kp - 1 and i == 1))
                lsbs, recs, bcss, ctxns = [], [], [], []
                for u in range(2):
                    lsbs.append(sm.tile([1, 512], f32, tag="lsb",
                                        name=f"lsb{u}"))
                    nc.vector.tensor_copy(lsbs[u][:], avs[u][64:65, :])
                for u in range(2):
                    recs.append(sm.tile([1, 512], f32, tag="rec",
                                        name=f"rec{u}"))
                    nc.vector.reciprocal_approx_fast(recs[u][:], lsbs[u][:])
                for u in range(2):
                    bcss.append(sm.tile([64, 512], f32, tag="bcs",
                                        name=f"bcs{u}"))
                    nc.gpsimd.partition_broadcast(bcss[u][:], recs[u][:])
                for u in range(2):
                    ctxn = sm.tile([64, 512], bf16, tag="ctxn",
                                   name=f"ctxn{u}")
                    ctxns.append(ctxn)
                    nc.vector.tensor_mul(ctxn[:], avs[u][0:64, :], bcss[u][:])
                    a2v = a2a_in[qb].rearrange("(j f) c -> f j c", f=192)
                    nc.sync.dma_start(
                        a2v[64 * pair:64 * (pair + 1), 4 * u:4 * u + 4, :],
                        ctxns[u].rearrange("p (q c) -> p q c", q=4))

            def emit_a2a(qb):
                nc.gpsimd.collective_compute(
                    "AllToAll", mybir.AluOpType.bypass,
                    ins=[a2a_in[qb][:]], outs=[a2a_out[qb][:]],
                    replica_groups=RG)

            def emit_outproj_blk(qb, blk, ag):
                if blk == 0:
                    nc.sync.dma_start(
                        ag[:], a2a_out[qb].rearrange("(o p) r -> p o r", p=128))
                if True:
                    osb = outp.tile([128, D], f32, tag="osb")
                    for nh in range(2):
                        po = mix_ps.tile([128, 384], f32, tag="mix", name="po")
                        n0 = nh * 384
                        for j in range(KSUB):
                            nc.tensor.matmul(po[:],
                                             lhsT=ag[:, blk * KSUB + j, :],
                                             rhs=wp[:, j, n0:n0 + 384],
                                             start=(j == 0), stop=False)
                        nc.tensor.matmul(po[:], lhsT=onesP[:],
                                         rhs=bp_sb[:, n0:n0 + 384],
                                         start=False, stop=True)
                        nc.vector.tensor_copy(osb[:, n0:n0 + 384], po[:])
                    nc.sync.dma_start(out_d[qb, blk], osb[:])

            # ---- software-pipelined emission ---------------------------
            # warmup collective: absorb ncfw first-call overhead during proj
            nc.sync.dma_start(warm_in[0:128, :], masks[:, 0, 0:8])
            nc.gpsimd.collective_compute(
                "AllToAll", mybir.AluOpType.bypass,
                ins=[warm_in[:]], outs=[warm_out[:]], replica_groups=RG)
            # prologue: everything attention qb0 needs
            emit_qk_chunk(0, 0)
            emit_qk_chunk(1, 0)
            for rt in range(4):
                emit_v_tile(0, rt)
                emit_v_tile(1, rt)

            # filler queue: (deadline=(qb,pair,kp), emit_fn), kept in
            # deadline order; before each k-pair all units due by then are
            # drained (hard ordering requirement: a unit must be emitted
            # before the attention that consumes its output), plus one unit
            # opportunistically per k-pair to spread PE filler.
            from collections import deque
            fq = deque()
            for rc in range(1, 4):
                for b in range(2):
                    for ct in range(3):
                        # qT/kT rows rc needed from (qb=rc, pair0, kp0)
                        fq.append(((rc, 0, 0), lambda b=b, rc=rc, ct=ct:
                                   emit_qk_ct(b, rc, ct)))
                for rt in range(4 * rc, 4 * rc + 4):
                    for b in range(2):
                        # vE row-tile rt consumed at kp=rt//2 of (qb=rc,pair0)
                        dl = (rc, 0, max(0, rt // 2 - 1))
                        fq.append((dl, lambda b=b, rt=rt: emit_v_tile(b, rt)))
            fq = deque(sorted(fq, key=lambda t: t[0]))

            def drain(n, due=None):
                k = 0
                while fq and (k < n or (due and fq[0][0] <= due)):
                    fq.popleft()[1]()
                    k += 1

            for qb in range(4):
                for pair in range(HL):
                    emit_attn_pair(qb, pair, lambda kp, qb=qb, pair=pair:
                                   drain(1, due=(qb, pair, kp)))
                    if qb >= 1 and pair == (0 if qb >= 2 else 1):
                        # previous chunk's out-proj; A2A(qb-1) done by now
                        ag = agp.tile([128, 2 * KSUB, 128], bf16, tag="ag",
                                      name=f"ag{qb}")
                        fq.append(((qb, pair, 98), lambda q=qb - 1, a=ag:
                                   emit_outproj_blk(q, 0, a)))
                        fq.append(((qb, pair, 99), lambda q=qb - 1, a=ag:
                                   emit_outproj_blk(q, 1, a)))
                emit_a2a(qb)
            drain(99)
            ag3 = agp.tile([128, 2 * KSUB, 128], bf16, tag="ag", name="ag3")
            emit_outproj_blk(3, 0, ag3)
            emit_outproj_blk(3, 1, ag3)

    nc.compile()
    return nc


def _get_nc():
    if "nc" not in _CACHE:
        _CACHE["nc"] = _build_nc()
    return _CACHE["nc"]


def _masks_np():
    k = np.arange(128)[:, None]
    q = np.arange(512)[None, :]
    tiles = [(q >= k + 128 * t) for t in range(4)]
    m = np.stack([np.concatenate([tiles[2 * o], tiles[2 * o + 1]], axis=1)
                  for o in range(2)])
    return m.astype(BF16)


def _prep_in_maps(x, Wq, Wk, Wv, Wp, bp):
    x = np.asarray(x, dtype=np.float32)
    mk = _masks_np()
    wp_full = np.asarray(Wp).astype(BF16)
    bp_row = np.asarray(bp, dtype=np.float32).reshape(1, D).astype(BF16)
    xT_bg = []
    for bg in range(2):
        xl = x[2 * bg:2 * bg + 2].reshape(R, D)
        xT_bg.append(np.ascontiguousarray(xl.T).astype(BF16))
    wqk_hg, wv_hg = [], []
    for hg in range(4):
        hs = slice(192 * hg, 192 * (hg + 1))
        wqk_hg.append(np.concatenate(
            [np.asarray(Wq)[:, hs], np.asarray(Wk)[:, hs]], axis=1).astype(BF16))
        wv_hg.append(np.asarray(Wv)[:, hs].astype(BF16))
    in_maps = []
    for c in range(N_CORES):
        bg, hg = c // 4, c % 4
        in_maps.append({
            "xT": xT_bg[bg],
            "wqk": wqk_hg[hg],
            "wv": wv_hg[hg],
            "wp": wp_full,
            "bp": bp_row,
            "mk": mk,
        })
    return in_maps


def kernel(x, Wq, Wk, Wv, Wp, bp):
    from concourse import bass_utils

    nc = _get_nc()
    in_maps = _prep_in_maps(x, Wq, Wk, Wv, Wp, bp)
    res = bass_utils.run_bass_kernel_spmd(nc, in_maps,
                                          core_ids=list(range(N_CORES)))
    out = np.empty((B, S, D), np.float32)
    for c in range(N_CORES):
        sh = res.results[c]["out"]  # [4 chunks, 2 blocks, 128, D]
        for qb in range(4):
            for blk in range(2):
                batch = 2 * blk + c // 4
                s0 = 512 * qb + 128 * (c % 4)
                out[batch, s0:s0 + 128] = sh[qb, blk]
    return out


# revision 19
# speedup vs baseline: 1.0988x; 1.0519x over previous
"""Multi-head causal attention (B=4,S=2048,D=768,H=12,HD=64) on 8 Trainium2 cores.

Sharding: 4-way head tensor-parallel (3 heads/core) x 2-way batch data-parallel
(2 batches/core).  Core c: batch group bg=c//4 (batches 2bg,2bg+1), head group
hg=c%4 (heads 3hg..3hg+2).

Per-core device program (SPMD; per-core differences come only from data):
  1. q/k projections emitted transposed (qT,kT: [64 head-dim partitions, rows]);
     v projection row-major with an appended ones column per head (softmax
     denominator rides along the AV matmul as psum row 64).
  2. Causal attention computed transposed: S_T[k,q] = kT.T @ qT, so P=exp(S_T)
     feeds AV directly with no P transpose.  Softmax skips the running max
     (scores are O(1) at this problem's scale; exp is mathematically identical
     to the reference since softmax is shift-invariant).  The two batches of a
     head run concurrently on the PE via 64-row tile packing.  AV accumulates
     ctxU_T[65, q512] = sum_k vE.T @ P_T (row 64 = denominator l).  Normalize:
     1/l via fast-approx DVE reciprocal, broadcast across partitions on GpSimd,
     one fused DVE multiply.
  3. Per 512-row q-block (x2 batches = 1024-row chunk): 8-core AllToAll (bf16,
     128-row shards) redistributes ctx so each core holds all 768 context
     features for its own 2x128 output rows; local projection with full Wp;
     bias via a K=1 ones-outer-product matmul.

Emission is software-pipelined: qk/v projection chunks are interleaved between
attention pairs as PE filler (keeps the PE dense so HAM stays at full clock),
and each chunk's out-projection is emitted one q-block later so the PE never
head-of-line blocks on the AllToAll round trip.

Host side only slices/casts/transposes inputs and concatenates output shards.
"""

import sys

if "/opt/trn_rl_repo" not in sys.path:
    sys.path.insert(0, "/opt/trn_rl_repo")

import numpy as np
import ml_dtypes

BF16 = ml_dtypes.bfloat16

B, S, D = 4, 2048, 768
H, HD = 12, 64
N_CORES = 8
BL = 2          # batches per core
HL = 3          # heads per core
R = BL * S      # 4096 rows per core
KSUB = D // 128  # 6

_CACHE = {}


def _build_nc():
    import concourse.bass as bass  # noqa: F401
    import concourse.tile as tile
    from concourse import bacc, mybir

    f32 = mybir.dt.float32
    bf16 = mybir.dt.bfloat16
    EXP = mybir.ActivationFunctionType.Exp

    nc = bacc.Bacc("TRN2", target_bir_lowering=False, debug=False,
                   num_devices=N_CORES)

    xT_d = nc.dram_tensor("xT", [D, R], bf16, kind="ExternalInput").ap()
    wqk_d = nc.dram_tensor("wqk", [D, 2 * HL * HD], bf16, kind="ExternalInput").ap()
    wv_d = nc.dram_tensor("wv", [D, HL * HD], bf16, kind="ExternalInput").ap()
    wp_d = nc.dram_tensor("wp", [D, D], bf16, kind="ExternalInput").ap()
    bp_d = nc.dram_tensor("bp", [1, D], bf16, kind="ExternalInput").ap()
    mk_d = nc.dram_tensor("mk", [2, 128, 1024], bf16, kind="ExternalInput").ap()
    out_d = nc.dram_tensor("out", [4, 2, 128, D], f32, kind="ExternalOutput").ap()

    RG = [[0, 1, 2, 3, 4, 5, 6, 7]]

    with tile.TileContext(nc) as tc:
        with tc.tile_pool(name="persist", bufs=1) as per, \
             tc.tile_pool(name="dram", bufs=1, space="DRAM") as dram, \
             tc.tile_pool(name="mix_ps", bufs=2, space="PSUM") as mix_ps, \
             tc.tile_pool(name="st_ps", bufs=2, space="PSUM") as st_ps, \
             tc.tile_pool(name="av_ps", bufs=2, space="PSUM") as av_ps, \
             tc.tile_pool(name="pt", bufs=8) as ptp, \
             tc.tile_pool(name="sm", bufs=4) as sm, \
             tc.tile_pool(name="agp", bufs=2) as agp, \
             tc.tile_pool(name="outp", bufs=2) as outp:
            # ---- persistent SBUF tensors -------------------------------
            wqk = per.tile([128, KSUB, 2 * HL * HD], bf16, tag="wqk")
            wv = per.tile([128, KSUB, HL * HD], bf16, tag="wv")
            xT = per.tile([128, KSUB, R], bf16, tag="xT")
            xTr = xT_d.rearrange("(o p) r -> p o r", p=128)
            for j in range(KSUB):  # consumption order for fast PE start
                nc.sync.dma_start(
                    wqk[:, j], wqk_d.rearrange("(o p) c -> p o c", p=128)[:, j])
                nc.sync.dma_start(xT[:, j, 0:512], xTr[:, j, 0:512])
                nc.sync.dma_start(
                    wv[:, j], wv_d.rearrange("(o p) c -> p o c", p=128)[:, j])
            for rc in range(1, 8):
                for j in range(KSUB):
                    nc.sync.dma_start(xT[:, j, rc * 512:rc * 512 + 512],
                                      xTr[:, j, rc * 512:rc * 512 + 512])
            masks = per.tile([128, 2, 1024], bf16, tag="mk")
            nc.sync.dma_start(masks[:], mk_d.rearrange("o p c -> p o c"))
            wp = per.tile([128, KSUB, D], bf16, tag="wp")
            nc.sync.dma_start(wp[:], wp_d.rearrange("(o p) c -> p o c", p=128))
            bp_sb = per.tile([1, D], bf16, tag="bp")
            nc.sync.dma_start(bp_sb[:], bp_d[:])
            onesP = per.tile([1, 128], bf16, tag="onesP")
            nc.vector.memset(onesP[:], 1.0)

            # pair p = head p; partitions 0:64 = batch 0, 64:128 = batch 1
            qT = per.tile([128, HL, S], bf16, tag="qT")
            kT = per.tile([128, HL, S], bf16, tag="kT")
            vE = per.tile([128, 2 * 16, HL * 65], bf16, tag="vE")
            nc.vector.memset(vE[:], 1.0)

            warm_in = dram.tile([8 * 192, 8], bf16, tag="warm_in")
            warm_out = dram.tile([8 * 192, 8], bf16, tag="warm_out")
            a2a_in = [dram.tile([8 * 192, 128], bf16, name=f"a2ai{qb}",
                                tag=f"a2ai{qb}") for qb in range(4)]
            a2a_out = [dram.tile([8 * 192, 128], bf16, name=f"a2ao{qb}",
                                 tag=f"a2ao{qb}") for qb in range(4)]

            # ---- emission helpers --------------------------------------
            def emit_qk_ct(b, rc, ct):
                r0 = b * S + rc * 512
                if True:
                    ps = mix_ps.tile([128, 512], f32, tag="mix", name="ps")
                    for j in range(KSUB):
                        nc.tensor.matmul(
                            ps[:],
                            lhsT=wqk[:, j, ct * 128:(ct + 1) * 128],
                            rhs=xT[:, j, r0:r0 + 512],
                            start=(j == 0), stop=(j == KSUB - 1))
                    for half in range(2):
                        gid = 2 * ct + half
                        dest = qT if gid < 3 else kT
                        pair = gid % 3
                        nc.vector.tensor_copy(
                            dest[b * 64:(b + 1) * 64, pair,
                                 rc * 512:(rc + 1) * 512],
                            ps[half * 64:(half + 1) * 64, :])

            def emit_qk_chunk(b, rc):
                for ct in range(3):
                    emit_qk_ct(b, rc, ct)

            def emit_v_tile(b, rt):
                r0 = b * S + rt * 128
                psv = mix_ps.tile([128, HL * HD], f32, tag="mix", name="psv")
                for j in range(KSUB):
                    nc.tensor.matmul(
                        psv[:], lhsT=xT[:, j, r0:r0 + 128], rhs=wv[:, j, :],
                        start=(j == 0), stop=(j == KSUB - 1))
                for h in range(HL):
                    nc.vector.tensor_copy(
                        vE[:, b * 16 + rt, h * 65:h * 65 + 64],
                        psv[:, h * 64:(h + 1) * 64])

            def emit_attn_pair(qb, pair, drain):
                n_k = 4 * (qb + 1)
                n_kp = n_k // 2
                q0 = qb * 512
                avs = [av_ps.tile([65, 512], f32, tag="av", name=f"av{u}")
                       for u in range(2)]
                for kp in range(n_kp):
                    drain(kp)  # due-units for this kp must precede it
                    o = kp - (n_kp - 2)  # diag pair offset; >=0 on diagonal
                    qv0 = 256 if o == 1 else 0  # valid q starts here
                    stps = [st_ps.tile([128, 2, 512], f32, tag="st",
                                       name=f"st{u}") for u in range(2)]
                    for i in range(2):
                        for u in range(2):
                            kt = 2 * kp + i
                            nc.tensor.matmul(
                                stps[u][:, i, qv0:512],
                                lhsT=kT[u * 64:(u + 1) * 64, pair,
                                        kt * 128:(kt + 1) * 128],
                                rhs=qT[u * 64:(u + 1) * 64, pair,
                                       q0 + qv0:q0 + 512],
                                start=True, stop=True)
                    for u in range(2):
                        pt = ptp.tile([128, 2, 512], bf16, tag="pt")
                        if qv0:
                            nc.vector.memset(pt[:, :, 0:qv0], 0.0)
                        nc.scalar.activation(pt[:, :, qv0:512],
                                             stps[u][:, :, qv0:512], EXP,
                                             scale=float(HD) ** -0.5)
                        if o >= 0:
                            mk2 = masks[:, o, :].rearrange("p (i c) -> p i c",
                                                           i=2)
                            nc.vector.tensor_mul(pt[:, :, qv0:512],
                                                 pt[:, :, qv0:512],
                                                 mk2[:, :, qv0:512])
                        for i in range(2):
                            kt = 2 * kp + i
                            nc.tensor.matmul(
                                avs[u][:],
                                lhsT=vE[:, u * 16 + kt,
                                        pair * 65:(pair + 1) * 65],
                                rhs=pt[:, i, :],
                                start=(kp == 0 and i == 0),
                                stop=(kp == n_kp - 1 and i == 1))
                lsbs, recs, bcss, ctxns = [], [], [], []
                for u in range(2):
                    lsbs.append(sm.tile([1, 512], f32, tag="lsb",
                                        name=f"lsb{u}"))
                    nc.vector.tensor_copy(lsbs[u][:], avs[u][64:65, :])
                for u in range(2):
                    recs.append(sm.tile([1, 512], f32, tag="rec",
                                        name=f"rec{u}"))
                    nc.vector.reciprocal_approx_fast(recs[u][:], lsbs[u][:])
                for u in range(2):
                    bcss.append(sm.tile([64, 512], f32, tag="bcs",
                                        name=f"bcs{u}"))
                    nc.gpsimd.partition_broadcast(bcss[u][:], recs[u][:])
                for u in range(2):
                    ctxn = sm.tile([64, 512], bf16, tag="ctxn",
                                   name=f"ctxn{u}")
                    ctxns.append(ctxn)
                    nc.vector.tensor_mul(ctxn[:], avs[u][0:64, :], bcss[u][:])
                    a2v = a2a_in[qb].rearrange("(j f) c -> f j c", f=192)
                    nc.sync.dma_start(
                        a2v[64 * pair:64 * (pair + 1), 4 * u:4 * u + 4, :],
                        ctxns[u].rearrange("p (q c) -> p q c", q=4))

            def emit_a2a(qb):
                nc.gpsimd.collective_compute(
                    "AllToAll", mybir.AluOpType.bypass,
                    ins=[a2a_in[qb][:]], outs=[a2a_out[qb][:]],
                    replica_groups=RG)

            def emit_outproj_blk(qb, blk, ag):
                if blk == 0:
                    nc.sync.dma_start(
                        ag[:], a2a_out[qb].rearrange("(o p) r -> p o r", p=128))
                if True:
                    osb = outp.tile([128, D], f32, tag="osb")
                    for nh in range(2):
                        po = mix_ps.tile([128, 384], f32, tag="mix", name="po")
                        n0 = nh * 384
                        for j in range(KSUB):
                            nc.tensor.matmul(po[:],
                                             lhsT=ag[:, blk * KSUB + j, :],
                                             rhs=wp[:, j, n0:n0 + 384],
                                             start=(j == 0), stop=False)
                        nc.tensor.matmul(po[:], lhsT=onesP[:],
                                         rhs=bp_sb[:, n0:n0 + 384],
                                         start=False, stop=True)
                        nc.vector.tensor_copy(osb[:, n0:n0 + 384], po[:])
                    nc.sync.dma_start(out_d[qb, blk], osb[:])

            # ---- software-pipelined emission ---------------------------
            # warmup collective: absorb ncfw first-call overhead during proj
            nc.sync.dma_start(warm_in[0:128, :], masks[:, 0, 0:8])
            nc.gpsimd.collective_compute(
                "AllToAll", mybir.AluOpType.bypass,
                ins=[warm_in[:]], outs=[warm_out[:]], replica_groups=RG)
            # prologue: everything attention qb0 needs
            emit_qk_chunk(0, 0)
            emit_qk_chunk(1, 0)
            for rt in range(4):
                emit_v_tile(0, rt)
                emit_v_tile(1, rt)

            # filler queue: (deadline=(qb,pair,kp), emit_fn), kept in
            # deadline order; before each k-pair all units due by then are
            # drained (hard ordering requirement: a unit must be emitted
            # before the attention that consumes its output), plus one unit
            # opportunistically per k-pair to spread PE filler.
            from collections import deque
            fq = deque()
            for rc in range(1, 4):
                for b in range(2):
                    for ct in range(3):
                        # qT/kT rows rc needed from (qb=rc, pair0, kp0)
                        fq.append(((rc, 0, 0), lambda b=b, rc=rc, ct=ct:
                                   emit_qk_ct(b, rc, ct)))
                for rt in range(4 * rc, 4 * rc + 4):
                    for b in range(2):
                        # vE row-tile rt consumed at kp=rt//2 of (qb=rc,pair0)
                        dl = (rc, 0, max(0, rt // 2 - 1))
                        fq.append((dl, lambda b=b, rt=rt: emit_v_tile(b, rt)))
            fq = deque(sorted(fq, key=lambda t: t[0]))

            def drain(n, due=None):
                k = 0
                while fq and (k < n or (due and fq[0][0] <= due)):
                    fq.popleft()[1]()
                    k += 1

            for qb in range(4):
                for pair in range(HL):
                    emit_attn_pair(qb, pair, lambda kp, qb=qb, pair=pair:
                                   drain(1, due=(qb, pair, kp)))
                    # out-proj of chunk q consumed ~2 chunks later so the
                    # big late q-blocks keep PE filler (A2A long done)
                    op_sched = {(2, 0): 0, (3, 0): 1, (3, 1): 2}
                    if (qb, pair) in op_sched:
                        q = op_sched[(qb, pair)]
                        ag = agp.tile([128, 2 * KSUB, 128], bf16, tag="ag",
                                      name=f"ag{q}")
                        fq.append(((qb, pair, 98), lambda q=q, a=ag:
                                   emit_outproj_blk(q, 0, a)))
                        fq.append(((qb, pair, 99), lambda q=q, a=ag:
                                   emit_outproj_blk(q, 1, a)))
                emit_a2a(qb)
            drain(99)
            ag3 = agp.tile([128, 2 * KSUB, 128], bf16, tag="ag", name="ag3")
            emit_outproj_blk(3, 0, ag3)
            emit_outproj_blk(3, 1, ag3)

    nc.compile()
    return nc


def _get_nc():
    if "nc" not in _CACHE:
        _CACHE["nc"] = _build_nc()
    return _CACHE["nc"]


def _masks_np():
    k = np.arange(128)[:, None]
    q = np.arange(512)[None, :]
    tiles = [(q >= k + 128 * t) for t in range(4)]
    m = np.stack([np.concatenate([tiles[2 * o], tiles[2 * o + 1]], axis=1)
                  for o in range(2)])
    return m.astype(BF16)


def _prep_in_maps(x, Wq, Wk, Wv, Wp, bp):
    x = np.asarray(x, dtype=np.float32)
    mk = _masks_np()
    wp_full = np.asarray(Wp).astype(BF16)
    bp_row = np.asarray(bp, dtype=np.float32).reshape(1, D).astype(BF16)
    xT_bg = []
    for bg in range(2):
        xl = x[2 * bg:2 * bg + 2].reshape(R, D)
        xT_bg.append(np.ascontiguousarray(xl.T).astype(BF16))
    wqk_hg, wv_hg = [], []
    for hg in range(4):
        hs = slice(192 * hg, 192 * (hg + 1))
        wqk_hg.append(np.concatenate(
            [np.asarray(Wq)[:, hs], np.asarray(Wk)[:, hs]], axis=1).astype(BF16))
        wv_hg.append(np.asarray(Wv)[:, hs].astype(BF16))
    in_maps = []
    for c in range(N_CORES):
        bg, hg = c // 4, c % 4
        in_maps.append({
            "xT": xT_bg[bg],
            "wqk": wqk_hg[hg],
            "wv": wv_hg[hg],
            "wp": wp_full,
            "bp": bp_row,
            "mk": mk,
        })
    return in_maps


def kernel(x, Wq, Wk, Wv, Wp, bp):
    from concourse import bass_utils

    nc = _get_nc()
    in_maps = _prep_in_maps(x, Wq, Wk, Wv, Wp, bp)
    res = bass_utils.run_bass_kernel_spmd(nc, in_maps,
                                          core_ids=list(range(N_CORES)))
    out = np.empty((B, S, D), np.float32)
    for c in range(N_CORES):
        sh = res.results[c]["out"]  # [4 chunks, 2 blocks, 128, D]
        for qb in range(4):
            for blk in range(2):
                batch = 2 * blk + c // 4
                s0 = 512 * qb + 128 * (c % 4)
                out[batch, s0:s0 + 128] = sh[qb, blk]
    return out
